# Initial kernel scaffold
#
"""CrossAttention (B=2, S=2048, D=1024, H=16, DH=64) on 8 TRN2 NeuronCores.

Megatron-style head sharding: core i owns heads {2i, 2i+1} (a 128-column
slice of Wq/Wk/Wv), computes attention for those heads over both batches,
gathers the per-head attention outputs across cores, then computes a
128-column slice of the output projection.

v2 schedule: one continuous emission stream. Projections, V-transposes and
output-projection matmuls are emitted as "filler" micro-units interleaved
into the attention kb loop so the PE never idles (keeps the tensor engine
at its max p-state). Softmax normalize runs entirely off the PE queue
(reciprocal_approx_fast on DVE + partition_broadcast on GpSimd).

Gather modes:
  cc   — AllGather collectives (baseline mechanism)
  rdma — direct SBUF->SBUF remote DMA to all peers + semaphore waits.
         Core r stores sender (r XOR j)'s block in slot j; the host
         permutes Wo kb-blocks per core to match.
"""
import os
import numpy as np
from collections import deque
from contextlib import ExitStack

import bass_rust
from concourse import bacc
import concourse.bass as bass
import concourse.mybir as mybir
import concourse.tile as tile
from concourse.bass_utils import run_bass_kernel_spmd
from concourse import library_config

F32 = mybir.dt.float32
BF16 = mybir.dt.bfloat16
MMDT = BF16

B, S, D = 2, 2048, 1024
H, DH = 16, 64
NCORES = 8
T = B * S                 # 4096 tokens
HPC = H // NCORES         # 2 heads per core
W_SL = HPC * DH           # 128
SCALE = DH ** -0.5        # 0.125
KB_D = D // 128           # 8 contraction blocks over D
QC = S // 512             # 4 query chunks per batch
KBS = S // 128            # 16 key blocks per batch
NCH = B * QC              # 8 chunks total

GATHER = os.environ.get("KERNEL_GATHER", "cc")   # "cc" | "rdma"
CC_SPLIT = [[2, 2], [3, 1]]

_NC_CACHE = {}


def build_nc():
    rdma = GATHER == "rdma"
    nc = bacc.Bacc(num_devices=NCORES)

    xt = nc.dram_tensor("xt", [D, T], MMDT, kind="ExternalInput")       # x^T
    wq = nc.dram_tensor("wq", [128, KB_D * W_SL], MMDT, kind="ExternalInput")
    wk = nc.dram_tensor("wk", [128, KB_D * W_SL], MMDT, kind="ExternalInput")
    wv = nc.dram_tensor("wv", [128, KB_D * W_SL], MMDT, kind="ExternalInput")
    wo = nc.dram_tensor("wo", [128, KB_D * W_SL], MMDT, kind="ExternalInput")
    bo = nc.dram_tensor("bo", [W_SL, 1], F32, kind="ExternalInput")
    out = nc.dram_tensor("out", [W_SL, T], F32, kind="ExternalOutput")  # out^T slice

    if not rdma:
        o_loc = [[nc.dram_tensor(f"o_loc{b}_{hf}", [W_SL, 512 * n], MMDT,
                                 kind="Internal")
                  for hf, n in enumerate(CC_SPLIT[b])] for b in range(B)]
        o_gat = [[nc.dram_tensor(f"o_gat{b}_{hf}", [NCORES * W_SL, 512 * n], MMDT,
                                 kind="Internal", addr_space="Shared")
                  for hf, n in enumerate(CC_SPLIT[b])] for b in range(B)]

    xt_r = xt.ap().rearrange("(kb p) t -> p kb t", p=128)

    # pre-TileContext: the barrier's matching increment is inserted at
    # compile time, so the wait must not be visible to the tile scheduler
    if rdma:
        nc.gpsimd.bir_kernel_barrier_wait([list(range(NCORES))])
        nc.gpsimd.load_library(library_config.proxy)
    pending_waits = []

    with tile.TileContext(nc) as tc, ExitStack() as ctx:
        wp = ctx.enter_context(tc.tile_pool(name="wp", bufs=1))
        xp = ctx.enter_context(tc.tile_pool(name="xp", bufs=4))
        ep = ctx.enter_context(tc.tile_pool(name="ep", bufs=6))
        npl = ctx.enter_context(tc.tile_pool(name="npl", bufs=2))
        opl = ctx.enter_context(tc.tile_pool(name="opl", bufs=2))
        ps = ctx.enter_context(tc.tile_pool(name="ps", bufs=2, space="PSUM"))

        if rdma:
            g_sems = [nc.alloc_semaphore(f"gsem{k}") for k in range(NCH)]
            l_sem = nc.alloc_semaphore("lsem")
        else:
            nc.gpsimd.load_library(library_config.proxy)

        # ---- static SBUF ----
        wq_sb = wp.tile([128, KB_D, W_SL], MMDT, name="wq_sb")
        wk_sb = wp.tile([128, KB_D, W_SL], MMDT, name="wk_sb")
        wv_sb = wp.tile([128, KB_D, W_SL], MMDT, name="wv_sb")
        wo_sb = wp.tile([128, KB_D, W_SL], MMDT, name="wo_sb")
        for w_sb, w_d in ((wq_sb, wq), (wk_sb, wk), (wv_sb, wv), (wo_sb, wo)):
            nc.sync.dma_start(out=w_sb, in_=w_d.ap().rearrange(
                "p (kb m) -> p kb m", kb=KB_D))
        bo_sb = wp.tile([W_SL, 1], F32, name="bo_sb")
        nc.sync.dma_start(out=bo_sb, in_=bo.ap())
        import ml_dtypes
        eye = np.eye(128, dtype=ml_dtypes.bfloat16)
        ident_d = nc.inline_tensor(eye, name="ident")
        ident = wp.tile([128, 128], MMDT, name="ident_sb")
        nc.sync.dma_start(out=ident, in_=ident_d.ap().bitcast(MMDT))

        qT = [wp.tile([128, S], MMDT, name=f"qT{b}") for b in range(B)]
        kT = [wp.tile([128, S], MMDT, name=f"kT{b}") for b in range(B)]
        vT = [wp.tile([128, S], MMDT, name=f"vT{b}") for b in range(B)]
        v_aug = [wp.tile([128, KBS, 130], MMDT, name=f"v_aug{b}") for b in range(B)]
        osb_st = wp.tile([128, NCH, 512], MMDT, name="osb_st")
        if rdma:
            og_sb = wp.tile([128, NCH * KB_D, 512], MMDT, name="og_sb")

        cc_insts = {}

        # ---------- generators ----------
        xc_tiles = {}

        def emit_x_dma(b, tcb):
            gsl = slice(b * S + tcb * 512, b * S + (tcb + 1) * 512)
            xc = xp.tile([128, KB_D, 512], MMDT, tag="xc", name="xc")
            nc.sync.dma_start(out=xc, in_=xt_r[:, :, gsl])
            xc_tiles[(b, tcb)] = xc

        def proj_group(b, tcb, w_sb, dst):
            """One projection group: 8 accumulating matmuls + copy out.
            Yields every 2 matmuls."""
            sl = slice(tcb * 512, (tcb + 1) * 512)
            xc = xc_tiles[(b, tcb)]
            acc = ps.tile([128, 512], F32, tag="pp", name="acc")
            for kb in range(KB_D):
                nc.tensor.matmul(acc, w_sb[:, kb, :], xc[:, kb, :],
                                 start=(kb == 0), stop=(kb == KB_D - 1))
                if kb % 2 == 1:
                    yield
            nc.vector.tensor_copy(dst[:, sl], acc)
            yield

        def vtrans_group(b, tcb):
            """Transpose V chunk tcb into v_aug natural layout."""
            for kb in range(4 * tcb, 4 * tcb + 4):
                tp = ps.tile([128, 512], MMDT, tag="pp", name="tp",
                             padded_shape=[128, 512])
                nc.tensor.transpose(tp[:, 0:128], vT[b][:, kb * 128:(kb + 1) * 128],
                                    ident)
                nc.vector.tensor_copy(v_aug[b][:, kb, 0:64], tp[:, 0:64])
                nc.vector.tensor_copy(v_aug[b][:, kb, 65:129], tp[:, 64:128])
                if kb % 2 == 1:
                    yield

        def proj_gen(b):
            # ones columns for the fused row-sum (both heads), once per batch
            nc.vector.memset(v_aug[b][:, :, 64:65], 1.0)
            nc.vector.memset(v_aug[b][:, :, 129:130], 1.0)
            staged = b == 1  # avoid early WAR-blocked x dmas for batch 1
            for tcb in range(2 if staged else QC):
                emit_x_dma(b, tcb)
            yield
            for tcb in range(QC):
                yield from proj_group(b, tcb, wk_sb, kT[b])
                if staged and tcb < 2:
                    emit_x_dma(b, tcb + 2)
            yield from proj_group(b, 0, wq_sb, qT[b])
            yield from proj_group(b, 0, wv_sb, vT[b])
            yield from vtrans_group(b, 0)
            yield "attn_ready"
            for tcb in range(1, QC):
                yield from proj_group(b, tcb, wv_sb, vT[b])
                yield from vtrans_group(b, tcb)
            for tcb in range(1, QC):
                yield from proj_group(b, tcb, wq_sb, qT[b])

        def outproj_gen(b, tcb):
            cci = b * QC + tcb
            qsl_g = slice(b * S + tcb * 512, b * S + (tcb + 1) * 512)
            if rdma:
                # ge-0 is trivially satisfied in the tile scheduling sim;
                # the real threshold (16) is patched in post-scheduling
                w = nc.tensor.wait_ge(g_sems[cci], 0)
                pending_waits.append((w, g_sems[cci]))
                og_sl = lambda kb: og_sb[:, cci * KB_D + kb, :]
            else:
                hf = 0 if tcb < CC_SPLIT[b][0] else 1
                off = tcb if hf == 0 else tcb - CC_SPLIT[b][0]
                og_r = o_gat[b][hf].ap().rearrange(
                    "(kb p) t -> p kb t", p=128)[:, :, off * 512:off * 512 + 512]
                ogt = xp.tile([128, KB_D, 512], MMDT, tag="og", name="ogt")
                g = nc.sync.dma_start(out=ogt, in_=og_r)
                bass_rust.add_dep_helper(g.ins, cc_insts[(b, hf)].ins,
                                         sync=True, reason="og after cc")
                og_sl = lambda kb: ogt[:, kb, :]
                yield
            accw = ps.tile([128, 512], F32, tag="pp", name="accw")
            for kb in range(KB_D):
                nc.tensor.matmul(accw, wo_sb[:, kb, :], og_sl(kb),
                                 start=(kb == 0), stop=(kb == KB_D - 1))
                if kb % 2 == 1:
                    yield
            osb2 = opl.tile([128, 512], F32, name="osb2")
            nc.vector.tensor_scalar_add(osb2, accw, bo_sb[:, 0:1])
            nc.sync.dma_start(out=out.ap()[:, qsl_g], in_=osb2)
            yield

        # ---------- scheduler ----------
        fillers = deque()

        def pull(n=1):
            for _ in range(n):
                while fillers:
                    try:
                        next(fillers[0])
                        break
                    except StopIteration:
                        fillers.popleft()
                else:
                    return

        def attn_chunk(b, qc):
            cci = b * QC + qc
            qsl = slice(qc * 512, (qc + 1) * 512)
            po = [ps.tile([65, 512], F32, tag="po1", bufs=1, name="po1"),
                  ps.tile([65, 512], F32, tag="po2", bufs=1, name="po2")]
            ets = {}

            def sc(kb):
                s_ps = ps.tile([128, 1024], F32, tag="aps", name="s_ps")
                ksl = slice(kb * 128, (kb + 1) * 128)
                for h in range(HPC):
                    hsl = slice(h * 64, (h + 1) * 64)
                    nc.tensor.matmul(
                        s_ps[:, h * 512:(h + 1) * 512],
                        kT[b][hsl, ksl], qT[b][hsl, qsl],
                        start=True, stop=True, tile_position=(h * 64, 0))
                et = ep.tile([128, 1024], MMDT, tag="et", name="et")
                nc.scalar.activation(out=et, in_=s_ps,
                                     func=mybir.ActivationFunctionType.Exp,
                                     scale=SCALE)
                ets[kb] = et

            def av(kb):
                et = ets.pop(kb)
                for h in range(HPC):
                    nc.tensor.matmul(
                        po[h][0:65, :],
                        v_aug[b][:, kb, h * 65:(h + 1) * 65],
                        et[:, h * 512:(h + 1) * 512],
                        start=(kb == 0), stop=(kb == KBS - 1))

            sc(0)
            pull(2)
            sc(1)
            pull(2)
            for kb in range(KBS):
                if kb + 2 < KBS:
                    sc(kb + 2)
                av(kb)
                pull(2)

            # normalize — entirely off the PE queue
            # reciprocal_approx_fast needs partition-base-0 input: stage the
            # PSUM sum rows (partition 64) into base-0 SBUF tiles first
            s0 = npl.tile([1, 512], F32, tag="srow", bufs=4, name="s0")
            s1 = npl.tile([1, 512], F32, tag="srow", bufs=4, name="s1")
            nc.vector.tensor_copy(s0, po[0][64:65, :])
            nc.vector.tensor_copy(s1, po[1][64:65, :])
            rec0 = npl.tile([1, 512], F32, tag="rec", bufs=4, name="rec0")
            rec1 = npl.tile([1, 512], F32, tag="rec", bufs=4, name="rec1")
            nc.vector.reciprocal_approx_fast(rec0, s0)
            nc.vector.reciprocal_approx_fast(rec1, s1)
            bcb = npl.tile([64, 1024], F32, tag="bc", name="bcb")
            nc.gpsimd.partition_broadcast(bcb[:, 0:512], rec0, channels=64)
            nc.gpsimd.partition_broadcast(bcb[:, 512:1024], rec1, channels=64)
            with nc.allow_low_precision(reason="softmax normalize"):
                nc.vector.tensor_mul(osb_st[0:64, cci, :], po[0][0:64, :],
                                     bcb[:, 0:512])
                nc.vector.tensor_mul(osb_st[64:128, cci, :], po[1][0:64, :],
                                     bcb[:, 512:1024])

            # ship the normalized chunk
            if rdma:
                for j in range(NCORES):
                    nc.gpsimd.remote_dma_broadcast(
                        out_ap=og_sb[:, cci * KB_D + j, :],
                        in_ap=osb_st[:, cci, :],
                        remote_sem=g_sems[cci], local_sem=l_sem,
                        rdests=[(0, j) if kk == j else None
                                for kk in range(NCORES)])
                nc.gpsimd.trigger_dma(count=None)
            else:
                hf = 0 if qc < CC_SPLIT[b][0] else 1
                off = qc if hf == 0 else qc - CC_SPLIT[b][0]
                d = nc.sync.dma_start(
                    out=o_loc[b][hf].ap()[:, off * 512:off * 512 + 512],
                    in_=osb_st[:, cci, :])
                if qc == CC_SPLIT[b][0] - 1 or qc == QC - 1:
                    cc = nc.gpsimd.collective_compute(
                        "AllGather", mybir.AluOpType.bypass,
                        replica_groups=[list(range(NCORES))],
                        ins=[o_loc[b][hf].ap()], outs=[o_gat[b][hf].ap()])
                    bass_rust.add_dep_helper(cc.ins, d.ins, sync=True,
                                             reason="cc after o_loc")
                    # previous chunk's o_loc dma of the same half
                    prev = getattr(attn_chunk, "_pending_oloc", [])
                    for dd in prev:
                        bass_rust.add_dep_helper(cc.ins, dd, sync=True,
                                                 reason="cc after o_loc")
                    attn_chunk._pending_oloc = []
                    cc_insts[(b, hf)] = cc
                else:
                    attn_chunk._pending_oloc = getattr(
                        attn_chunk, "_pending_oloc", []) + [d.ins]

        # prologue: b0 projections up to attn-ready, then chunk-major loop
        pg0, pg1 = proj_gen(0), proj_gen(1)
        for m in pg0:
            if m == "attn_ready":
                break
        fillers.append(pg0)

        for b in range(B):
            for qc in range(QC):
                attn_chunk(b, qc)
                if b == 0 and qc == 0:
                    fillers.append(pg1)
                if rdma:
                    if b == 1 and qc == 0:
                        fillers.append(outproj_gen(0, 0))
                        fillers.append(outproj_gen(0, 1))
                    if b == 1 and qc == 1:
                        fillers.append(outproj_gen(0, 2))
                        fillers.append(outproj_gen(0, 3))
                    if b == 1 and qc == 2:
                        fillers.append(outproj_gen(1, 0))
                    if b == 1 and qc == 3:
                        fillers.append(outproj_gen(1, 1))
                else:
                    if b == 1 and qc == 0:
                        fillers.append(outproj_gen(0, 0))
                        fillers.append(outproj_gen(0, 1))
                    if b == 1 and qc == 2:
                        fillers.append(outproj_gen(0, 2))
                        fillers.append(outproj_gen(0, 3))

        # drain remaining fillers, then the tail out-projections
        while fillers:
            pull()
        tail = [(1, 2), (1, 3)] if rdma else [(1, 0), (1, 1), (1, 2), (1, 3)]
        for b, tcb in tail:
            for _ in outproj_gen(b, tcb):
                pass

    # patch the real remote-gather thresholds onto the waits, now that the
    # tile scheduler (which cannot model cross-core increments) has run
    for w, sem in pending_waits:
        w.wait_op(sem, 16, "sem-ge", check=False)

    nc.finalize()
    return nc


def _tile_w(w, np_dt):
    # [D, W_SL] -> [128, KB_D*W_SL] matching sbuf tile [128, kb, m]
    return np.ascontiguousarray(
        w.reshape(KB_D, 128, W_SL).transpose(1, 0, 2).reshape(128, KB_D * W_SL)
    ).astype(np_dt)


def kernel(x, Wq, Wk, Wv, Wo, bo):
    import ml_dtypes
    np_dt = ml_dtypes.bfloat16
    x = np.asarray(x, dtype=np.float32)
    Wq = np.asarray(Wq, dtype=np.float32)
    Wk = np.asarray(Wk, dtype=np.float32)
    Wv = np.asarray(Wv, dtype=np.float32)
    Wo = np.asarray(Wo, dtype=np.float32)
    bo = np.asarray(bo, dtype=np.float32)

    if "nc" not in _NC_CACHE:
        _NC_CACHE["nc"] = build_nc()
    nc = _NC_CACHE["nc"]

    xt = np.ascontiguousarray(x.reshape(T, D).T).astype(np_dt)  # [D, T]
    in_maps = []
    for c in range(NCORES):
        csl = slice(c * W_SL, (c + 1) * W_SL)
        if GATHER == "rdma":
            # slot j on core c holds sender (c XOR j)'s o-dim block
            wo_rows = np.concatenate(
                [Wo[(c ^ j) * 128:((c ^ j) + 1) * 128, csl] for j in range(NCORES)],
                axis=0)
        else:
            wo_rows = Wo[:, csl]
        in_maps.append({
            "xt": xt,
            "wq": _tile_w(Wq[:, csl], np_dt),
            "wk": _tile_w(Wk[:, csl], np_dt),
            "wv": _tile_w(Wv[:, csl], np_dt),
            "wo": _tile_w(wo_rows, np_dt),
            "bo": np.ascontiguousarray(bo[csl]).reshape(W_SL, 1),
        })
    res = run_bass_kernel_spmd(nc, in_maps, core_ids=list(range(NCORES)))
    LAST_RESULT["exec_time_ns"] = res.exec_time_ns
    LAST_RESULT["scope_times"] = res.per_core_scope_times
    LAST_RESULT["trace"] = res.instructions_and_trace[1] if res.instructions_and_trace else None
    out_t = np.concatenate([res.results[c]["out"] for c in range(NCORES)], axis=0)
    return np.ascontiguousarray(out_t.T).reshape(B, S, D)


LAST_RESULT = {}



# revision 41
# speedup vs baseline: 1.0826x; 1.0826x over previous
"""CrossAttention (B=2, S=2048, D=1024, H=16, DH=64) on 8 TRN2 NeuronCores.

Megatron-style head sharding: core i owns heads {2i, 2i+1} (a 128-column
slice of Wq/Wk/Wv), computes attention for those heads over both batches,
gathers the per-head attention outputs across cores (per-chunk AllGather),
then computes a 128-column slice of the output projection.

v4 schedule:
  - One continuous emission stream; projections / V-transposes / output
    projections are filler micro-units interleaved into the attention loop.
  - AV matmuls are COLUMN-TILED pairs: head0 -> PE cols 0-63 (psum
    partitions 0-63), head1 -> cols 64-127, running concurrently.
  - Softmax denominators: exp tiles accumulate off-PE into two fp16
    accumulators (DVE: even kb, GpSimd: odd kb), then 4 tiny ones^T@acc
    matmuls (M=1, N=512) produce the denominators in PSUM partition 0.
  - Per-chunk AllGather (8 collectives) so only the final chunk's gather
    is exposed at the tail; outproj(i) is filler during chunk i+2.
  - Static-tile RAW hazards (attention reading kT/qT/v2 slices that filler
    projections write mid-stream) are closed with explicit sync deps: the
    tile framework does not emit cross-engine semaphores for those pairs.
"""
import numpy as np
from collections import deque
from contextlib import ExitStack

import bass_rust
from concourse import bacc
import concourse.bass as bass
import concourse.mybir as mybir
import concourse.tile as tile
from concourse.bass_utils import run_bass_kernel_spmd
from concourse import library_config

F32 = mybir.dt.float32
BF16 = mybir.dt.bfloat16
F16 = mybir.dt.float16
MMDT = BF16

B, S, D = 2, 2048, 1024
H, DH = 16, 64
NCORES = 8
T = B * S                 # 4096 tokens
HPC = H // NCORES         # 2 heads per core
W_SL = HPC * DH           # 128
SCALE = DH ** -0.5        # 0.125
KB_D = D // 128           # 8 contraction blocks over D
QC = S // 512             # 4 query chunks per batch
KBS = S // 128            # 16 key blocks per batch
NCH = B * QC              # 8 chunks total

_NC_CACHE = {}


def build_nc():
    nc = bacc.Bacc(num_devices=NCORES)

    xt = nc.dram_tensor("xt", [D, T], MMDT, kind="ExternalInput")       # x^T
    # first x chunk pre-tiled on host: contiguous per partition, so its DMA
    # descriptor generation (which gates PE start) is trivial
    xt0 = nc.dram_tensor("xt0", [128, KB_D * 512], MMDT, kind="ExternalInput")
    wq = nc.dram_tensor("wq", [128, KB_D * W_SL], MMDT, kind="ExternalInput")
    wk = nc.dram_tensor("wk", [128, KB_D * W_SL], MMDT, kind="ExternalInput")
    wv = nc.dram_tensor("wv", [128, KB_D * W_SL], MMDT, kind="ExternalInput")
    wo = nc.dram_tensor("wo", [128, KB_D * W_SL], MMDT, kind="ExternalInput")
    bo = nc.dram_tensor("bo", [W_SL, 1], F32, kind="ExternalInput")
    out = nc.dram_tensor("out", [W_SL, T], F32, kind="ExternalOutput")  # out^T slice

    # gather groups: chunk indices per AllGather. Two-chunk gathers finish
    # ~2 chunks after their data is ready (ring + cross-core skew), so each
    # out-projection can run as filler 2+ chunks later without stalling.
    GATHERS = [[0, 1], [2, 3], [4, 5], [6], [7]]
    g_of = {}
    for gi, chs in enumerate(GATHERS):
        for off, cci in enumerate(chs):
            g_of[cci] = (gi, off)
    o_loc = [nc.dram_tensor(f"o_loc{g}", [W_SL, 512 * len(chs)], MMDT,
                            kind="Internal")
             for g, chs in enumerate(GATHERS)]
    o_gat = [nc.dram_tensor(f"o_gat{g}", [NCORES * W_SL, 512 * len(chs)], MMDT,
                            kind="Internal", addr_space="Shared")
             for g, chs in enumerate(GATHERS)]

    xt_r = xt.ap().rearrange("(kb p) t -> p kb t", p=128)

    with tile.TileContext(nc) as tc, ExitStack() as ctx:
        wp = ctx.enter_context(tc.tile_pool(name="wp", bufs=1))
        xp = ctx.enter_context(tc.tile_pool(name="xp", bufs=4))
        op2 = ctx.enter_context(tc.tile_pool(name="op2", bufs=4))
        ep = ctx.enter_context(tc.tile_pool(name="ep", bufs=6))
        dp = ctx.enter_context(tc.tile_pool(name="dp", bufs=2))
        npl = ctx.enter_context(tc.tile_pool(name="npl", bufs=2))
        opl = ctx.enter_context(tc.tile_pool(name="opl", bufs=2))
        ps = ctx.enter_context(tc.tile_pool(name="ps", bufs=2, space="PSUM"))

        nc.gpsimd.load_library(library_config.proxy)

        # ---- static SBUF ----
        wk_sb = wp.tile([128, KB_D, W_SL], MMDT, name="wk_sb")
        wq_sb = wp.tile([128, KB_D, W_SL], MMDT, name="wq_sb")
        wv_sb = wp.tile([128, KB_D, W_SL], MMDT, name="wv_sb")
        wo_sb = wp.tile([128, KB_D, W_SL], MMDT, name="wo_sb")

        xc_tiles = {}

        def emit_x_dma(b, tcb):
            xc = xp.tile([128, KB_D, 512], MMDT, tag="xc", name="xc")
            if (b, tcb) == (0, 0):
                src = xt0.ap().rearrange("p (kb t) -> p kb t", kb=KB_D)
            else:
                gsl = slice(b * S + tcb * 512, b * S + (tcb + 1) * 512)
                src = xt_r[:, :, gsl]
            nc.sync.dma_start(out=xc, in_=src)
            xc_tiles[(b, tcb)] = xc

        def w_dma(w_sb, w_d):
            nc.sync.dma_start(out=w_sb, in_=w_d.ap().rearrange(
                "p (kb m) -> p kb m", kb=KB_D))

        # wk + x chunk 0 first: they gate the first projection group (PE
        # start). The DMA engines are bandwidth-bound here, so everything
        # not needed immediately (wo/bo/ident, later x chunks) is deferred
        # into the projection stream.
        w_dma(wk_sb, wk)
        emit_x_dma(0, 0)
        w_dma(wq_sb, wq)
        w_dma(wv_sb, wv)
        bo_sb = wp.tile([W_SL, 1], F32, name="bo_sb")
        import ml_dtypes
        eye = np.eye(128, dtype=ml_dtypes.bfloat16)
        ident_d = nc.inline_tensor(eye, name="ident")
        ident = wp.tile([128, 128], MMDT, name="ident_sb")
        ones_sb = wp.tile([128, 1], F16, name="ones_sb")
        nc.vector.memset(ones_sb, 1.0)

        def emit_late_dmas():
            w_dma(wo_sb, wo)
            nc.sync.dma_start(out=bo_sb, in_=bo.ap())
            nc.sync.dma_start(out=ident, in_=ident_d.ap().bitcast(MMDT))

        qT = [wp.tile([128, S], MMDT, name=f"qT{b}") for b in range(B)]
        kT = [wp.tile([128, S], MMDT, name=f"kT{b}") for b in range(B)]
        vT = [wp.tile([128, S], MMDT, name=f"vT{b}") for b in range(B)]
        # v2[b][:, kb, :]: [128 keys, 128] — cols 0-63 head0 dims, 64-127 head1
        v2 = [wp.tile([128, KBS, 128], F16, name=f"v2_{b}") for b in range(B)]
        osb_st = wp.tile([128, NCH, 512], MMDT, name="osb_st")

        cc_insts = {}

        # RAW-hazard guard: producer instruction + emission chunk index per
        # static-tile slice; consumers within 1 chunk add an explicit dep.
        prod = {}
        cur_ci = [-1]

        def record(key, inst):
            prod[key] = (inst, cur_ci[0])

        def guard(mm, key):
            ent = prod.get(key)
            if ent is not None:
                bass_rust.add_dep_helper(mm.ins, ent[0].ins, sync=True,
                                         reason=f"raw {key}")

        # ---------- generators ----------
        def proj_group(b, tcb, w_sb, dst, key):
            """One projection group: 8 accumulating matmuls + copy out."""
            sl = slice(tcb * 512, (tcb + 1) * 512)
            xc = xc_tiles[(b, tcb)]
            acc = ps.tile([128, 512], F32, tag="pp", name="acc")
            for kb in range(KB_D):
                nc.tensor.matmul(acc, w_sb[:, kb, :], xc[:, kb, :],
                                 start=(kb == 0), stop=(kb == KB_D - 1))
                if kb % 2 == 1:
                    yield
            cp = nc.vector.tensor_copy(dst[:, sl], acc)
            record((key, b, tcb), cp)
            yield

        def vtrans_group(b, tcb):
            """Transpose V chunk tcb into v2 [keys, dims] layout (fp16)."""
            for kb in range(4 * tcb, 4 * tcb + 4):
                tp = ps.tile([128, 512], MMDT, tag="pp", name="tp",
                             padded_shape=[128, 512])
                tr = nc.tensor.transpose(tp[:, 0:128],
                                         vT[b][:, kb * 128:(kb + 1) * 128],
                                         ident)
                guard(tr, ('v', b, tcb))
                cp = nc.vector.tensor_copy(v2[b][:, kb, :], tp[:, 0:128])
                record(('v2', b, kb), cp)
                if kb % 2 == 1:
                    yield

        def proj_gen(b):
            if b == 1:
                emit_x_dma(b, 0)
                yield
            yield from proj_group(b, 0, wk_sb, kT[b], 'k')
            emit_x_dma(b, 1)
            yield from proj_group(b, 1, wk_sb, kT[b], 'k')
            emit_x_dma(b, 2)
            yield from proj_group(b, 2, wk_sb, kT[b], 'k')
            emit_x_dma(b, 3)
            if b == 0:
                emit_late_dmas()
            yield from proj_group(b, 3, wk_sb, kT[b], 'k')
            yield from proj_group(b, 0, wq_sb, qT[b], 'q')
            yield from proj_group(b, 0, wv_sb, vT[b], 'v')
            yield from vtrans_group(b, 0)
            yield "attn_ready"
            # V-chain first: v2 blocks for kb>=4 must be EMITTED before the
            # attention avs that consume them (consumer-after-producer program
            # order is required; deps alone cannot fix emission order).
            for tcb in range(1, QC):
                yield from proj_group(b, tcb, wv_sb, vT[b], 'v')
                yield from vtrans_group(b, tcb)
            for tcb in range(1, QC):
                yield from proj_group(b, tcb, wq_sb, qT[b], 'q')

        def outproj_gen(cci, anchor=None):
            b, tcb = divmod(cci, QC)
            gi, off = g_of[cci]
            qsl_g = slice(b * S + tcb * 512, b * S + (tcb + 1) * 512)
            og_r = o_gat[gi].ap().rearrange("(kb p) t -> p kb t", p=128)[
                :, :, off * 512:(off + 1) * 512]
            ogt = op2.tile([128, KB_D, 512], MMDT, tag="og", name="ogt")
            g = nc.sync.dma_start(out=ogt, in_=og_r)
            bass_rust.add_dep_helper(g.ins, cc_insts[gi].ins,
                                     sync=True, reason="og after cc")
            yield
            accw = ps.tile([128, 512], F32, tag="pp", name="accw")
            for kb in range(KB_D):
                mm = nc.tensor.matmul(accw, wo_sb[:, kb, :], ogt[:, kb, :],
                                      start=(kb == 0), stop=(kb == KB_D - 1))
                if kb == 0 and anchor is not None:
                    # ordering-only anchor: keep the scheduler from hoisting
                    # these gather-dependent matmuls into earlier attention
                    # chunks, where an unmet collective dep stalls the
                    # in-order PE queue (cross-core skew is unmodeled).
                    bass_rust.add_dep_helper(mm.ins, anchor.ins, sync=False,
                                             reason="outproj anchor")
                if kb % 2 == 1:
                    yield
            osb2 = opl.tile([128, 512], F32, name="osb2")
            nc.vector.tensor_scalar_add(osb2, accw, bo_sb[:, 0:1])
            nc.sync.dma_start(out=out.ap()[:, qsl_g], in_=osb2)
            yield

        # ---------- scheduler ----------
        fillers = deque()
        pending_epi = [None]
        gather_deps = {}
        last_av = {}

        def pull(n=1):
            for _ in range(n):
                while fillers:
                    try:
                        next(fillers[0])
                        break
                    except StopIteration:
                        fillers.popleft()
                else:
                    return

        def attn_chunk(b, qc):
            cci = b * QC + qc
            qsl = slice(qc * 512, (qc + 1) * 512)
            po = ps.tile([128, 512], F32, tag="po", bufs=2, name="po")
            acc_v = dp.tile([128, 1024], F16, tag="av", name="acc_v")
            ets = {}

            def sc(kb):
                s_ps = ps.tile([128, 1024], F32, tag="aps", name="s_ps")
                ksl = slice(kb * 128, (kb + 1) * 128)
                for h in range(HPC):
                    hsl = slice(h * 64, (h + 1) * 64)
                    mm = nc.tensor.matmul(
                        s_ps[:, h * 512:(h + 1) * 512],
                        kT[b][hsl, ksl], qT[b][hsl, qsl],
                        start=True, stop=True, tile_position=(h * 64, 0))
                    if h == 0:
                        guard(mm, ('k', b, kb // 4))
                        guard(mm, ('q', b, qc))
                et = ep.tile([128, 1024], F16, tag="et", name="et")
                nc.scalar.activation(out=et, in_=s_ps,
                                     func=mybir.ActivationFunctionType.Exp,
                                     scale=SCALE)
                ets[kb] = et

            def av(kb):
                et = ets[kb]
                mm = nc.tensor.matmul(po[0:64, :], v2[b][:, kb, 0:64],
                                      et[:, 0:512],
                                      start=(kb == 0), stop=(kb == KBS - 1),
                                      tile_position=(0, 0))
                guard(mm, ('v2', b, kb))
                mm2 = nc.tensor.matmul(po[64:128, :], v2[b][:, kb, 64:128],
                                       et[:, 512:1024],
                                       start=(kb == 0), stop=(kb == KBS - 1),
                                       tile_position=(0, 64))
                last_av[cci] = mm2

            def dacc(kb):
                # denominator accumulation, DVE only: GpSimd must stay out
                # of the attention flow — a collective trigger blocks its
                # queue while the CC core is busy, and any attention-coupled
                # GpSimd op would stall et recycling behind it. Emitted one
                # kb late so its wait on exp(kb) never head-of-line-blocks
                # the DVE FIFO.
                et = ets.pop(kb)
                if kb == 0:
                    nc.vector.tensor_copy(acc_v, et)
                else:
                    nc.vector.tensor_add(acc_v, acc_v, et)

            sc(0)
            pull(2)
            sc(1)
            # previous chunk's epilogue: its waits (denominator fold on DVE,
            # recip, broadcasts) now overlap this chunk's fresh work instead
            # of stalling the in-order PE queue at the boundary.
            if pending_epi[0] is not None:
                pending_epi[0]()
                pending_epi[0] = None
            pull(2)
            for kb in range(KBS):
                if kb + 2 < KBS:
                    sc(kb + 2)
                av(kb)
                if kb >= 1:
                    dacc(kb - 1)
                # chunks 0-1 must pull hard: b0's filler V-chain/qT groups
                # have to be EMITTED before the avs/scores that read them.
                pull((2 if kb < 12 else 1) if cci < 2 else 1)
            dacc(KBS - 1)

            def epilogue():
                # The dummy aps allocation keeps the next chunk's sc(0) off
                # the buffer the recip is still reading.
                ps.tile([128, 1024], F32, tag="aps", name="dummy")
                d_ps = ps.tile([128, 1024], F32, tag="aps", name="d_ps")
                for half in range(2):
                    hsl = slice(half * 512, (half + 1) * 512)
                    nc.tensor.matmul(d_ps[0:1, hsl], ones_sb, acc_v[:, hsl],
                                     start=True, stop=True)
                rec = npl.tile([1, 1024], F32, tag="rec", bufs=2, name="rec")
                nc.vector.reciprocal_approx_fast(rec, d_ps[0:1, :])
                bcb = npl.tile([64, 1024], F32, tag="bc", name="bcb")
                nc.gpsimd.partition_broadcast(bcb[:, 0:512], rec[0:1, 0:512],
                                              channels=64)
                nc.gpsimd.partition_broadcast(bcb[:, 512:1024],
                                              rec[0:1, 512:1024], channels=64)
                with nc.allow_low_precision(reason="softmax normalize"):
                    nc.vector.tensor_mul(osb_st[0:64, cci, :], po[0:64, :],
                                         bcb[:, 0:512])
                    nc.vector.tensor_mul(osb_st[64:128, cci, :],
                                         po[64:128, :], bcb[:, 512:1024])

                gi, off = g_of[cci]
                d = nc.sync.dma_start(
                    out=o_loc[gi].ap()[:, off * 512:(off + 1) * 512],
                    in_=osb_st[:, cci, :])
                gather_deps.setdefault(gi, []).append(d.ins)
                if off == len(GATHERS[gi]) - 1:
                    cc = nc.gpsimd.collective_compute(
                        "AllGather", mybir.AluOpType.bypass,
                        replica_groups=[list(range(NCORES))],
                        ins=[o_loc[gi].ap()], outs=[o_gat[gi].ap()])
                    for dd in gather_deps[gi]:
                        bass_rust.add_dep_helper(cc.ins, dd, sync=True,
                                                 reason="cc after o_loc")
                    cc_insts[gi] = cc

            pending_epi[0] = epilogue

        # prologue: b0 projections up to attn-ready, then chunk-major loop
        emit_x_dma(0, 0)
        pg0, pg1 = proj_gen(0), proj_gen(1)
        for m in pg0:
            if m == "attn_ready":
                break
        fillers.append(pg0)

        for ci in range(NCH):
            cur_ci[0] = ci
            b, qc = divmod(ci, QC)
            attn_chunk(b, qc)
            if ci == 0:
                fillers.append(pg1)
            if ci == 4:
                fillers.append(outproj_gen(0, anchor=last_av[4]))
                fillers.append(outproj_gen(1, anchor=last_av[4]))
            if ci == 5:
                fillers.append(outproj_gen(2, anchor=last_av[5]))
                fillers.append(outproj_gen(3, anchor=last_av[5]))
            if ci == 7:
                fillers.append(outproj_gen(4, anchor=last_av[7]))
                fillers.append(outproj_gen(5, anchor=last_av[7]))

        # final chunk epilogue, drain fillers, then the tail out-projections.
        # Both tail ogt DMAs are issued first so their gather waits overlap
        # the other out-projections' matmuls.
        cur_ci[0] = NCH
        pending_epi[0]()
        pending_epi[0] = None
        op6 = outproj_gen(NCH - 2, anchor=last_av[7])
        next(op6)
        op7 = outproj_gen(NCH - 1, anchor=last_av[7])
        next(op7)
        while fillers:
            pull()
        for _ in op6:
            pass
        for _ in op7:
            pass

    nc.finalize()
    return nc


def _tile_w(w, np_dt):
    # [D, W_SL] -> [128, KB_D*W_SL] matching sbuf tile [128, kb, m]
    return np.ascontiguousarray(
        w.reshape(KB_D, 128, W_SL).transpose(1, 0, 2).reshape(128, KB_D * W_SL)
    ).astype(np_dt)


def kernel(x, Wq, Wk, Wv, Wo, bo):
    import ml_dtypes
    np_dt = ml_dtypes.bfloat16
    x = np.asarray(x, dtype=np.float32)
    Wq = np.asarray(Wq, dtype=np.float32)
    Wk = np.asarray(Wk, dtype=np.float32)
    Wv = np.asarray(Wv, dtype=np.float32)
    Wo = np.asarray(Wo, dtype=np.float32)
    bo = np.asarray(bo, dtype=np.float32)

    if "nc" not in _NC_CACHE:
        _NC_CACHE["nc"] = build_nc()
    nc = _NC_CACHE["nc"]

    xt = np.ascontiguousarray(x.reshape(T, D).T).astype(np_dt)  # [D, T]
    xt0 = np.ascontiguousarray(
        xt[:, 0:512].reshape(KB_D, 128, 512).transpose(1, 0, 2).reshape(
            128, KB_D * 512))
    in_maps = []
    for c in range(NCORES):
        csl = slice(c * W_SL, (c + 1) * W_SL)
        in_maps.append({
            "xt": xt,
            "xt0": xt0,
            "wq": _tile_w(Wq[:, csl], np_dt),
            "wk": _tile_w(Wk[:, csl], np_dt),
            "wv": _tile_w(Wv[:, csl], np_dt),
            "wo": _tile_w(Wo[:, csl], np_dt),
            "bo": np.ascontiguousarray(bo[csl]).reshape(W_SL, 1),
        })
    res = run_bass_kernel_spmd(nc, in_maps, core_ids=list(range(NCORES)))
    LAST_RESULT["exec_time_ns"] = res.exec_time_ns
    LAST_RESULT["scope_times"] = res.per_core_scope_times
    LAST_RESULT["trace"] = res.instructions_and_trace[1] if res.instructions_and_trace else None
    out_t = np.concatenate([res.results[c]["out"] for c in range(NCORES)], axis=0)
    return np.ascontiguousarray(out_t.T).reshape(B, S, D)


LAST_RESULT = {}


# revision 45
# speedup vs baseline: 1.0914x; 1.0081x over previous
"""CrossAttention (B=2, S=2048, D=1024, H=16, DH=64) on 8 TRN2 NeuronCores.

Megatron-style head sharding: core i owns heads {2i, 2i+1} (a 128-column
slice of Wq/Wk/Wv), computes attention for those heads over both batches,
gathers the per-head attention outputs across cores (per-chunk AllGather),
then computes a 128-column slice of the output projection.

v4 schedule:
  - One continuous emission stream; projections / V-transposes / output
    projections are filler micro-units interleaved into the attention loop.
  - AV matmuls are COLUMN-TILED pairs: head0 -> PE cols 0-63 (psum
    partitions 0-63), head1 -> cols 64-127, running concurrently.
  - Softmax denominators: exp tiles accumulate off-PE into two fp16
    accumulators (DVE: even kb, GpSimd: odd kb), then 4 tiny ones^T@acc
    matmuls (M=1, N=512) produce the denominators in PSUM partition 0.
  - Per-chunk AllGather (8 collectives) so only the final chunk's gather
    is exposed at the tail; outproj(i) is filler during chunk i+2.
  - Static-tile RAW hazards (attention reading kT/qT/v2 slices that filler
    projections write mid-stream) are closed with explicit sync deps: the
    tile framework does not emit cross-engine semaphores for those pairs.
"""
import numpy as np
from collections import deque
from contextlib import ExitStack

import bass_rust
from concourse import bacc
import concourse.bass as bass
import concourse.mybir as mybir
import concourse.tile as tile
from concourse.bass_utils import run_bass_kernel_spmd
from concourse import library_config

F32 = mybir.dt.float32
BF16 = mybir.dt.bfloat16
F16 = mybir.dt.float16
MMDT = BF16

B, S, D = 2, 2048, 1024
H, DH = 16, 64
NCORES = 8
T = B * S                 # 4096 tokens
HPC = H // NCORES         # 2 heads per core
W_SL = HPC * DH           # 128
SCALE = DH ** -0.5        # 0.125
KB_D = D // 128           # 8 contraction blocks over D
QC = S // 512             # 4 query chunks per batch
KBS = S // 128            # 16 key blocks per batch
NCH = B * QC              # 8 chunks total

_NC_CACHE = {}


def build_nc():
    nc = bacc.Bacc(num_devices=NCORES)

    xt = nc.dram_tensor("xt", [D, T], MMDT, kind="ExternalInput")       # x^T
    # first x chunk pre-tiled on host: contiguous per partition, so its DMA
    # descriptor generation (which gates PE start) is trivial
    xt0 = nc.dram_tensor("xt0", [128, KB_D * 512], MMDT, kind="ExternalInput")
    wq = nc.dram_tensor("wq", [128, KB_D * W_SL], MMDT, kind="ExternalInput")
    wk = nc.dram_tensor("wk", [128, KB_D * W_SL], MMDT, kind="ExternalInput")
    wv = nc.dram_tensor("wv", [128, KB_D * W_SL], MMDT, kind="ExternalInput")
    wo = nc.dram_tensor("wo", [128, KB_D * W_SL], MMDT, kind="ExternalInput")
    bo = nc.dram_tensor("bo", [W_SL, 1], F32, kind="ExternalInput")
    out = nc.dram_tensor("out", [W_SL, T], F32, kind="ExternalOutput")  # out^T slice

    # gather groups: chunk indices per AllGather. Two-chunk gathers finish
    # ~2 chunks after their data is ready (ring + cross-core skew), so each
    # out-projection can run as filler 2+ chunks later without stalling.
    GATHERS = [[0, 1], [2, 3], [4, 5], [6], [7]]
    g_of = {}
    for gi, chs in enumerate(GATHERS):
        for off, cci in enumerate(chs):
            g_of[cci] = (gi, off)
    o_loc = [nc.dram_tensor(f"o_loc{g}", [W_SL, 512 * len(chs)], MMDT,
                            kind="Internal")
             for g, chs in enumerate(GATHERS)]
    o_gat = [nc.dram_tensor(f"o_gat{g}", [NCORES * W_SL, 512 * len(chs)], MMDT,
                            kind="Internal", addr_space="Shared")
             for g, chs in enumerate(GATHERS)]

    xt_r = xt.ap().rearrange("(kb p) t -> p kb t", p=128)

    with tile.TileContext(nc) as tc, ExitStack() as ctx:
        wp = ctx.enter_context(tc.tile_pool(name="wp", bufs=1))
        xp = ctx.enter_context(tc.tile_pool(name="xp", bufs=4))
        op2 = ctx.enter_context(tc.tile_pool(name="op2", bufs=8))
        ep = ctx.enter_context(tc.tile_pool(name="ep", bufs=6))
        dp = ctx.enter_context(tc.tile_pool(name="dp", bufs=2))
        npl = ctx.enter_context(tc.tile_pool(name="npl", bufs=2))
        opl = ctx.enter_context(tc.tile_pool(name="opl", bufs=2))
        ps = ctx.enter_context(tc.tile_pool(name="ps", bufs=2, space="PSUM"))

        nc.gpsimd.load_library(library_config.proxy)

        # ---- static SBUF ----
        wk_sb = wp.tile([128, KB_D, W_SL], MMDT, name="wk_sb")
        wq_sb = wp.tile([128, KB_D, W_SL], MMDT, name="wq_sb")
        wv_sb = wp.tile([128, KB_D, W_SL], MMDT, name="wv_sb")
        wo_sb = wp.tile([128, KB_D, W_SL], MMDT, name="wo_sb")

        xc_tiles = {}

        def emit_x_dma(b, tcb):
            xc = xp.tile([128, KB_D, 512], MMDT, tag="xc", name="xc")
            if (b, tcb) == (0, 0):
                src = xt0.ap().rearrange("p (kb t) -> p kb t", kb=KB_D)
            else:
                gsl = slice(b * S + tcb * 512, b * S + (tcb + 1) * 512)
                src = xt_r[:, :, gsl]
            nc.sync.dma_start(out=xc, in_=src)
            xc_tiles[(b, tcb)] = xc

        def w_dma(w_sb, w_d):
            nc.sync.dma_start(out=w_sb, in_=w_d.ap().rearrange(
                "p (kb m) -> p kb m", kb=KB_D))

        # wk + x chunk 0 first: they gate the first projection group (PE
        # start). The DMA engines are bandwidth-bound here, so everything
        # not needed immediately (wo/bo/ident, later x chunks) is deferred
        # into the projection stream.
        w_dma(wk_sb, wk)
        emit_x_dma(0, 0)
        w_dma(wq_sb, wq)
        w_dma(wv_sb, wv)
        bo_sb = wp.tile([W_SL, 1], F32, name="bo_sb")
        import ml_dtypes
        eye = np.eye(128, dtype=ml_dtypes.bfloat16)
        ident_d = nc.inline_tensor(eye, name="ident")
        ident = wp.tile([128, 128], MMDT, name="ident_sb")
        ones_sb = wp.tile([128, 1], F16, name="ones_sb")
        nc.vector.memset(ones_sb, 1.0)

        def emit_late_dmas():
            w_dma(wo_sb, wo)
            nc.sync.dma_start(out=bo_sb, in_=bo.ap())
            nc.sync.dma_start(out=ident, in_=ident_d.ap().bitcast(MMDT))

        qT = [wp.tile([128, S], MMDT, name=f"qT{b}") for b in range(B)]
        kT = [wp.tile([128, S], MMDT, name=f"kT{b}") for b in range(B)]
        vT = [wp.tile([128, S], MMDT, name=f"vT{b}") for b in range(B)]
        # v2[b][:, kb, :]: [128 keys, 128] — cols 0-63 head0 dims, 64-127 head1
        v2 = [wp.tile([128, KBS, 128], F16, name=f"v2_{b}") for b in range(B)]
        osb_st = wp.tile([128, NCH, 512], MMDT, name="osb_st")

        cc_insts = {}

        # RAW-hazard guard: producer instruction + emission chunk index per
        # static-tile slice; consumers within 1 chunk add an explicit dep.
        prod = {}
        cur_ci = [-1]

        def record(key, inst):
            prod[key] = (inst, cur_ci[0])

        def guard(mm, key):
            ent = prod.get(key)
            if ent is not None:
                bass_rust.add_dep_helper(mm.ins, ent[0].ins, sync=True,
                                         reason=f"raw {key}")

        # ---------- generators ----------
        def proj_group(b, tcb, w_sb, dst, key):
            """One projection group: 8 accumulating matmuls + copy out."""
            sl = slice(tcb * 512, (tcb + 1) * 512)
            xc = xc_tiles[(b, tcb)]
            acc = ps.tile([128, 512], F32, tag="pp", name="acc")
            for kb in range(KB_D):
                nc.tensor.matmul(acc, w_sb[:, kb, :], xc[:, kb, :],
                                 start=(kb == 0), stop=(kb == KB_D - 1))
                if kb % 2 == 1:
                    yield
            cp = nc.vector.tensor_copy(dst[:, sl], acc)
            record((key, b, tcb), cp)
            yield

        def vtrans_group(b, tcb):
            """Transpose V chunk tcb into v2 [keys, dims] layout (fp16)."""
            for kb in range(4 * tcb, 4 * tcb + 4):
                tp = ps.tile([128, 512], MMDT, tag="pp", name="tp",
                             padded_shape=[128, 512])
                tr = nc.tensor.transpose(tp[:, 0:128],
                                         vT[b][:, kb * 128:(kb + 1) * 128],
                                         ident)
                guard(tr, ('v', b, tcb))
                cp = nc.vector.tensor_copy(v2[b][:, kb, :], tp[:, 0:128])
                record(('v2', b, kb), cp)
                if kb % 2 == 1:
                    yield

        def proj_gen(b):
            if b == 1:
                emit_x_dma(b, 0)
                yield
            yield from proj_group(b, 0, wk_sb, kT[b], 'k')
            emit_x_dma(b, 1)
            yield from proj_group(b, 1, wk_sb, kT[b], 'k')
            emit_x_dma(b, 2)
            yield from proj_group(b, 2, wk_sb, kT[b], 'k')
            emit_x_dma(b, 3)
            if b == 0:
                emit_late_dmas()
            yield from proj_group(b, 3, wk_sb, kT[b], 'k')
            yield from proj_group(b, 0, wq_sb, qT[b], 'q')
            yield from proj_group(b, 0, wv_sb, vT[b], 'v')
            yield from vtrans_group(b, 0)
            yield "attn_ready"
            # V-chain first: v2 blocks for kb>=4 must be EMITTED before the
            # attention avs that consume them (consumer-after-producer program
            # order is required; deps alone cannot fix emission order).
            for tcb in range(1, QC):
                yield from proj_group(b, tcb, wv_sb, vT[b], 'v')
                yield from vtrans_group(b, tcb)
            for tcb in range(1, QC):
                yield from proj_group(b, tcb, wq_sb, qT[b], 'q')

        ogt_tiles = {}

        def ogt_dma(cci):
            # prefetch the gathered attention output the moment its
            # collective is issued; the DMA waits off the PE queue.
            gi, off = g_of[cci]
            og_r = o_gat[gi].ap().rearrange("(kb p) t -> p kb t", p=128)[
                :, :, off * 512:(off + 1) * 512]
            ogt = op2.tile([128, KB_D, 512], MMDT, tag="og", name="ogt")
            g = nc.sync.dma_start(out=ogt, in_=og_r)
            bass_rust.add_dep_helper(g.ins, cc_insts[gi].ins,
                                     sync=True, reason="og after cc")
            ogt_tiles[cci] = ogt

        def outproj_mms(cci, anchor):
            b, tcb = divmod(cci, QC)
            qsl_g = slice(b * S + tcb * 512, b * S + (tcb + 1) * 512)
            ogt = ogt_tiles[cci]
            accw = ps.tile([128, 512], F32, tag="pp", name="accw")
            for kb in range(KB_D):
                mm = nc.tensor.matmul(accw, wo_sb[:, kb, :], ogt[:, kb, :],
                                      start=(kb == 0), stop=(kb == KB_D - 1))
                if kb == 0 and anchor is not None:
                    # ordering-only anchor: keep the scheduler from hoisting
                    # these gather-dependent matmuls into the attention
                    # stream, where an unmet collective dep would stall the
                    # in-order PE queue (cross-core skew is unmodeled).
                    bass_rust.add_dep_helper(mm.ins, anchor.ins, sync=False,
                                             reason="outproj anchor")
            osb2 = opl.tile([128, 512], F32, name="osb2")
            nc.vector.tensor_scalar_add(osb2, accw, bo_sb[:, 0:1])
            nc.sync.dma_start(out=out.ap()[:, qsl_g], in_=osb2)

        # ---------- scheduler ----------
        fillers = deque()
        pending_epi = [None]
        gather_deps = {}
        last_av = {}

        def pull(n=1):
            for _ in range(n):
                while fillers:
                    try:
                        next(fillers[0])
                        break
                    except StopIteration:
                        fillers.popleft()
                else:
                    return

        def attn_chunk(b, qc):
            cci = b * QC + qc
            qsl = slice(qc * 512, (qc + 1) * 512)
            po = ps.tile([128, 512], F32, tag="po", bufs=2, name="po")
            acc_v = dp.tile([128, 1024], F16, tag="av", name="acc_v")
            ets = {}

            def sc(kb):
                s_ps = ps.tile([128, 1024], F32, tag="aps", name="s_ps")
                ksl = slice(kb * 128, (kb + 1) * 128)
                for h in range(HPC):
                    hsl = slice(h * 64, (h + 1) * 64)
                    mm = nc.tensor.matmul(
                        s_ps[:, h * 512:(h + 1) * 512],
                        kT[b][hsl, ksl], qT[b][hsl, qsl],
                        start=True, stop=True, tile_position=(h * 64, 0))
                    if h == 0:
                        guard(mm, ('k', b, kb // 4))
                        guard(mm, ('q', b, qc))
                et = ep.tile([128, 1024], F16, tag="et", name="et")
                nc.scalar.activation(out=et, in_=s_ps,
                                     func=mybir.ActivationFunctionType.Exp,
                                     scale=SCALE)
                ets[kb] = et

            def av(kb):
                et = ets[kb]
                mm = nc.tensor.matmul(po[0:64, :], v2[b][:, kb, 0:64],
                                      et[:, 0:512],
                                      start=(kb == 0), stop=(kb == KBS - 1),
                                      tile_position=(0, 0))
                guard(mm, ('v2', b, kb))
                mm2 = nc.tensor.matmul(po[64:128, :], v2[b][:, kb, 64:128],
                                       et[:, 512:1024],
                                       start=(kb == 0), stop=(kb == KBS - 1),
                                       tile_position=(0, 64))
                last_av[cci] = mm2

            def dacc(kb):
                # denominator accumulation, DVE only: GpSimd must stay out
                # of the attention flow — a collective trigger blocks its
                # queue while the CC core is busy, and any attention-coupled
                # GpSimd op would stall et recycling behind it. Emitted one
                # kb late so its wait on exp(kb) never head-of-line-blocks
                # the DVE FIFO.
                et = ets.pop(kb)
                if kb == 0:
                    nc.vector.tensor_copy(acc_v, et)
                else:
                    nc.vector.tensor_add(acc_v, acc_v, et)

            sc(0)
            pull(2)
            sc(1)
            # previous chunk's epilogue: its waits (denominator fold on DVE,
            # recip, broadcasts) now overlap this chunk's fresh work instead
            # of stalling the in-order PE queue at the boundary.
            if pending_epi[0] is not None:
                pending_epi[0]()
                pending_epi[0] = None
            pull(2)
            for kb in range(KBS):
                if kb + 2 < KBS:
                    sc(kb + 2)
                av(kb)
                if kb >= 1:
                    dacc(kb - 1)
                # chunks 0-1 must pull hard: b0's filler V-chain/qT groups
                # have to be EMITTED before the avs/scores that read them.
                pull((2 if kb < 12 else 1) if cci < 2 else 1)
            dacc(KBS - 1)

            def epilogue():
                # The dummy aps allocation keeps the next chunk's sc(0) off
                # the buffer the recip is still reading.
                ps.tile([128, 1024], F32, tag="aps", name="dummy")
                d_ps = ps.tile([128, 1024], F32, tag="aps", name="d_ps")
                for half in range(2):
                    hsl = slice(half * 512, (half + 1) * 512)
                    nc.tensor.matmul(d_ps[0:1, hsl], ones_sb, acc_v[:, hsl],
                                     start=True, stop=True)
                rec = npl.tile([1, 1024], F32, tag="rec", bufs=2, name="rec")
                nc.vector.reciprocal_approx_fast(rec, d_ps[0:1, :])
                bcb = npl.tile([64, 1024], F32, tag="bc", name="bcb")
                nc.gpsimd.partition_broadcast(bcb[:, 0:512], rec[0:1, 0:512],
                                              channels=64)
                nc.gpsimd.partition_broadcast(bcb[:, 512:1024],
                                              rec[0:1, 512:1024], channels=64)
                with nc.allow_low_precision(reason="softmax normalize"):
                    nc.vector.tensor_mul(osb_st[0:64, cci, :], po[0:64, :],
                                         bcb[:, 0:512])
                    nc.vector.tensor_mul(osb_st[64:128, cci, :],
                                         po[64:128, :], bcb[:, 512:1024])

                gi, off = g_of[cci]
                d = nc.sync.dma_start(
                    out=o_loc[gi].ap()[:, off * 512:(off + 1) * 512],
                    in_=osb_st[:, cci, :])
                gather_deps.setdefault(gi, []).append(d.ins)
                if off == len(GATHERS[gi]) - 1:
                    cc = nc.gpsimd.collective_compute(
                        "AllGather", mybir.AluOpType.bypass,
                        replica_groups=[list(range(NCORES))],
                        ins=[o_loc[gi].ap()], outs=[o_gat[gi].ap()])
                    for dd in gather_deps[gi]:
                        bass_rust.add_dep_helper(cc.ins, dd, sync=True,
                                                 reason="cc after o_loc")
                    cc_insts[gi] = cc
                    for c2 in GATHERS[gi]:
                        ogt_dma(c2)

            pending_epi[0] = epilogue

        # prologue: b0 projections up to attn-ready, then chunk-major loop
        emit_x_dma(0, 0)
        pg0, pg1 = proj_gen(0), proj_gen(1)
        for m in pg0:
            if m == "attn_ready":
                break
        fillers.append(pg0)

        for ci in range(NCH):
            cur_ci[0] = ci
            b, qc = divmod(ci, QC)
            attn_chunk(b, qc)
            if ci == 0:
                fillers.append(pg1)

        # final chunk epilogue, drain fillers, then all out-projections as a
        # tail anchored after the last attention block: early chunks' data
        # is long gathered, and the last chunks' matmuls overlap the final
        # gathers' flight.
        cur_ci[0] = NCH
        pending_epi[0]()
        pending_epi[0] = None
        while fillers:
            pull()
        for cci in range(NCH):
            outproj_mms(cci, anchor=last_av[NCH - 1])

    nc.finalize()
    return nc


def _tile_w(w, np_dt):
    # [D, W_SL] -> [128, KB_D*W_SL] matching sbuf tile [128, kb, m]
    return np.ascontiguousarray(
        w.reshape(KB_D, 128, W_SL).transpose(1, 0, 2).reshape(128, KB_D * W_SL)
    ).astype(np_dt)


def kernel(x, Wq, Wk, Wv, Wo, bo):
    import ml_dtypes
    np_dt = ml_dtypes.bfloat16
    x = np.asarray(x, dtype=np.float32)
    Wq = np.asarray(Wq, dtype=np.float32)
    Wk = np.asarray(Wk, dtype=np.float32)
    Wv = np.asarray(Wv, dtype=np.float32)
    Wo = np.asarray(Wo, dtype=np.float32)
    bo = np.asarray(bo, dtype=np.float32)

    if "nc" not in _NC_CACHE:
        _NC_CACHE["nc"] = build_nc()
    nc = _NC_CACHE["nc"]

    xt = np.ascontiguousarray(x.reshape(T, D).T).astype(np_dt)  # [D, T]
    xt0 = np.ascontiguousarray(
        xt[:, 0:512].reshape(KB_D, 128, 512).transpose(1, 0, 2).reshape(
            128, KB_D * 512))
    in_maps = []
    for c in range(NCORES):
        csl = slice(c * W_SL, (c + 1) * W_SL)
        in_maps.append({
            "xt": xt,
            "xt0": xt0,
            "wq": _tile_w(Wq[:, csl], np_dt),
            "wk": _tile_w(Wk[:, csl], np_dt),
            "wv": _tile_w(Wv[:, csl], np_dt),
            "wo": _tile_w(Wo[:, csl], np_dt),
            "bo": np.ascontiguousarray(bo[csl]).reshape(W_SL, 1),
        })
    res = run_bass_kernel_spmd(nc, in_maps, core_ids=list(range(NCORES)))
    LAST_RESULT["exec_time_ns"] = res.exec_time_ns
    LAST_RESULT["scope_times"] = res.per_core_scope_times
    LAST_RESULT["trace"] = res.instructions_and_trace[1] if res.instructions_and_trace else None
    out_t = np.concatenate([res.results[c]["out"] for c in range(NCORES)], axis=0)
    return np.ascontiguousarray(out_t.T).reshape(B, S, D)


LAST_RESULT = {}


# revision 50
# speedup vs baseline: 1.1735x; 1.0752x over previous
"""CrossAttention (B=2, S=2048, D=1024, H=16, DH=64) on 8 TRN2 NeuronCores.

Megatron-style head sharding: core i owns heads {2i, 2i+1} (a 128-column
slice of Wq/Wk/Wv), computes attention for those heads over both batches,
gathers the per-head attention outputs across cores (per-chunk AllGather),
then computes a 128-column slice of the output projection.

v4 schedule:
  - One continuous emission stream; projections / V-transposes / output
    projections are filler micro-units interleaved into the attention loop.
  - AV matmuls are COLUMN-TILED pairs: head0 -> PE cols 0-63 (psum
    partitions 0-63), head1 -> cols 64-127, running concurrently.
  - Softmax denominators: exp tiles accumulate off-PE into two fp16
    accumulators (DVE: even kb, GpSimd: odd kb), then 4 tiny ones^T@acc
    matmuls (M=1, N=512) produce the denominators in PSUM partition 0.
  - Per-chunk AllGather (8 collectives) so only the final chunk's gather
    is exposed at the tail; outproj(i) is filler during chunk i+2.
  - Static-tile RAW hazards (attention reading kT/qT/v2 slices that filler
    projections write mid-stream) are closed with explicit sync deps: the
    tile framework does not emit cross-engine semaphores for those pairs.
"""
import numpy as np
from collections import deque
from contextlib import ExitStack

import bass_rust
from concourse import bacc
import concourse.bass as bass
import concourse.mybir as mybir
import concourse.tile as tile
from concourse.bass_utils import run_bass_kernel_spmd
from concourse import library_config

F32 = mybir.dt.float32
BF16 = mybir.dt.bfloat16
F16 = mybir.dt.float16
MMDT = BF16

B, S, D = 2, 2048, 1024
H, DH = 16, 64
NCORES = 8
T = B * S                 # 4096 tokens
HPC = H // NCORES         # 2 heads per core
W_SL = HPC * DH           # 128
SCALE = DH ** -0.5        # 0.125
KB_D = D // 128           # 8 contraction blocks over D
QC = S // 512             # 4 query chunks per batch
KBS = S // 128            # 16 key blocks per batch
NCH = B * QC              # 8 chunks total

_NC_CACHE = {}


def build_nc():
    nc = bacc.Bacc(num_devices=NCORES)

    xt = nc.dram_tensor("xt", [D, T], MMDT, kind="ExternalInput")       # x^T
    # first x chunk pre-tiled on host: contiguous per partition, so its DMA
    # descriptor generation (which gates PE start) is trivial
    xt0 = nc.dram_tensor("xt0", [128, KB_D * 512], MMDT, kind="ExternalInput")
    wq = nc.dram_tensor("wq", [128, KB_D * W_SL], MMDT, kind="ExternalInput")
    wk = nc.dram_tensor("wk", [128, KB_D * W_SL], MMDT, kind="ExternalInput")
    wv = nc.dram_tensor("wv", [128, KB_D * W_SL], MMDT, kind="ExternalInput")
    wo = nc.dram_tensor("wo", [128, KB_D * W_SL], MMDT, kind="ExternalInput")
    bo = nc.dram_tensor("bo", [W_SL, 1], F32, kind="ExternalInput")
    out = nc.dram_tensor("out", [W_SL, T], F32, kind="ExternalOutput")  # out^T slice

    # gather groups: chunk indices per AllGather. Two-chunk gathers finish
    # ~2 chunks after their data is ready (ring + cross-core skew), so each
    # out-projection can run as filler 2+ chunks later without stalling.
    GATHERS = [[0, 1], [2, 3], [4, 5], [6, 7]]
    g_of = {}
    for gi, chs in enumerate(GATHERS):
        for off, cci in enumerate(chs):
            g_of[cci] = (gi, off)
    o_loc = [nc.dram_tensor(f"o_loc{g}", [W_SL, 512 * len(chs)], MMDT,
                            kind="Internal")
             for g, chs in enumerate(GATHERS)]
    o_gat = [nc.dram_tensor(f"o_gat{g}", [NCORES * W_SL, 512 * len(chs)], MMDT,
                            kind="Internal", addr_space="Shared")
             for g, chs in enumerate(GATHERS)]

    xt_r = xt.ap().rearrange("(kb p) t -> p kb t", p=128)

    with tile.TileContext(nc) as tc, ExitStack() as ctx:
        wp = ctx.enter_context(tc.tile_pool(name="wp", bufs=1))
        xp = ctx.enter_context(tc.tile_pool(name="xp", bufs=4))
        op2 = ctx.enter_context(tc.tile_pool(name="op2", bufs=8))
        ep = ctx.enter_context(tc.tile_pool(name="ep", bufs=6))
        dp = ctx.enter_context(tc.tile_pool(name="dp", bufs=2))
        npl = ctx.enter_context(tc.tile_pool(name="npl", bufs=2))
        opl = ctx.enter_context(tc.tile_pool(name="opl", bufs=2))
        ps = ctx.enter_context(tc.tile_pool(name="ps", bufs=2, space="PSUM"))

        nc.gpsimd.load_library(library_config.proxy)

        # ---- static SBUF ----
        wk_sb = wp.tile([128, KB_D, W_SL], MMDT, name="wk_sb")
        wq_sb = wp.tile([128, KB_D, W_SL], MMDT, name="wq_sb")
        wv_sb = wp.tile([128, KB_D, W_SL], MMDT, name="wv_sb")
        wo_sb = wp.tile([128, KB_D, W_SL], MMDT, name="wo_sb")

        xc_tiles = {}

        def emit_x_dma(b, tcb):
            xc = xp.tile([128, KB_D, 512], MMDT, tag="xc", name="xc")
            if (b, tcb) == (0, 0):
                src = xt0.ap().rearrange("p (kb t) -> p kb t", kb=KB_D)
            else:
                gsl = slice(b * S + tcb * 512, b * S + (tcb + 1) * 512)
                src = xt_r[:, :, gsl]
            nc.sync.dma_start(out=xc, in_=src)
            xc_tiles[(b, tcb)] = xc

        def w_dma(w_sb, w_d):
            nc.sync.dma_start(out=w_sb, in_=w_d.ap().rearrange(
                "p (kb m) -> p kb m", kb=KB_D))

        # wk + x chunk 0 first: they gate the first projection group (PE
        # start). The DMA engines are bandwidth-bound here, so everything
        # not needed immediately (wo/bo/ident, later x chunks) is deferred
        # into the projection stream.
        w_dma(wk_sb, wk)
        emit_x_dma(0, 0)
        w_dma(wq_sb, wq)
        w_dma(wv_sb, wv)
        bo_sb = wp.tile([W_SL, 1], F32, name="bo_sb")
        import ml_dtypes
        eye = np.eye(128, dtype=ml_dtypes.bfloat16)
        ident_d = nc.inline_tensor(eye, name="ident")
        ident = wp.tile([128, 128], MMDT, name="ident_sb")
        ones_sb = wp.tile([128, 1], F16, name="ones_sb")
        nc.vector.memset(ones_sb, 1.0)

        def emit_late_dmas():
            w_dma(wo_sb, wo)
            nc.sync.dma_start(out=bo_sb, in_=bo.ap())
            nc.sync.dma_start(out=ident, in_=ident_d.ap().bitcast(MMDT))

        qT = [wp.tile([128, S], MMDT, name=f"qT{b}") for b in range(B)]
        kT = [wp.tile([128, S], MMDT, name=f"kT{b}") for b in range(B)]
        vT = [wp.tile([128, S], MMDT, name=f"vT{b}") for b in range(B)]
        # v2[b][:, kb, :]: [128 keys, 128] — cols 0-63 head0 dims, 64-127 head1
        v2 = [wp.tile([128, KBS, 128], F16, name=f"v2_{b}") for b in range(B)]
        osb_st = wp.tile([128, NCH, 512], MMDT, name="osb_st")

        cc_insts = {}

        # RAW-hazard guard: producer instruction + emission chunk index per
        # static-tile slice; consumers within 1 chunk add an explicit dep.
        prod = {}
        cur_ci = [-1]

        def record(key, inst):
            prod[key] = (inst, cur_ci[0])

        def guard(mm, key):
            ent = prod.get(key)
            if ent is not None:
                bass_rust.add_dep_helper(mm.ins, ent[0].ins, sync=True,
                                         reason=f"raw {key}")

        # ---------- generators ----------
        def proj_group(b, tcb, w_sb, dst, key):
            """One projection group: 8 accumulating matmuls + copy out."""
            sl = slice(tcb * 512, (tcb + 1) * 512)
            xc = xc_tiles[(b, tcb)]
            acc = ps.tile([128, 512], F32, tag="pp", name="acc")
            for kb in range(KB_D):
                nc.tensor.matmul(acc, w_sb[:, kb, :], xc[:, kb, :],
                                 start=(kb == 0), stop=(kb == KB_D - 1))
                if kb % 2 == 1:
                    yield
            cp = nc.vector.tensor_copy(dst[:, sl], acc)
            record((key, b, tcb), cp)
            yield

        def vtrans_group(b, tcb):
            """Transpose V chunk tcb into v2 [keys, dims] layout (fp16)."""
            for kb in range(4 * tcb, 4 * tcb + 4):
                tp = ps.tile([128, 512], MMDT, tag="pp", name="tp",
                             padded_shape=[128, 512])
                tr = nc.tensor.transpose(tp[:, 0:128],
                                         vT[b][:, kb * 128:(kb + 1) * 128],
                                         ident)
                guard(tr, ('v', b, tcb))
                cp = nc.vector.tensor_copy(v2[b][:, kb, :], tp[:, 0:128])
                record(('v2', b, kb), cp)
                if kb % 2 == 1:
                    yield

        def proj_gen(b):
            if b == 1:
                emit_x_dma(b, 0)
                yield
            yield from proj_group(b, 0, wk_sb, kT[b], 'k')
            emit_x_dma(b, 1)
            yield from proj_group(b, 1, wk_sb, kT[b], 'k')
            emit_x_dma(b, 2)
            yield from proj_group(b, 2, wk_sb, kT[b], 'k')
            emit_x_dma(b, 3)
            if b == 0:
                emit_late_dmas()
            yield from proj_group(b, 3, wk_sb, kT[b], 'k')
            yield from proj_group(b, 0, wq_sb, qT[b], 'q')
            yield from proj_group(b, 0, wv_sb, vT[b], 'v')
            yield from vtrans_group(b, 0)
            yield "attn_ready"
            # V-chain first: v2 blocks for kb>=4 must be EMITTED before the
            # attention avs that consume them (consumer-after-producer program
            # order is required; deps alone cannot fix emission order).
            for tcb in range(1, QC):
                yield from proj_group(b, tcb, wv_sb, vT[b], 'v')
                yield from vtrans_group(b, tcb)
            for tcb in range(1, QC):
                yield from proj_group(b, tcb, wq_sb, qT[b], 'q')

        ogt_tiles = {}

        def ogt_dma(cci):
            # prefetch the gathered attention output the moment its
            # collective is issued; the DMA waits off the PE queue.
            gi, off = g_of[cci]
            og_r = o_gat[gi].ap().rearrange("(kb p) t -> p kb t", p=128)[
                :, :, off * 512:(off + 1) * 512]
            ogt = op2.tile([128, KB_D, 512], MMDT, tag="og", name="ogt")
            g = nc.sync.dma_start(out=ogt, in_=og_r)
            bass_rust.add_dep_helper(g.ins, cc_insts[gi].ins,
                                     sync=True, reason="og after cc")
            ogt_tiles[cci] = ogt

        def outproj_mms(cci, anchor):
            b, tcb = divmod(cci, QC)
            qsl_g = slice(b * S + tcb * 512, b * S + (tcb + 1) * 512)
            ogt = ogt_tiles[cci]
            accw = ps.tile([128, 512], F32, tag="pp", name="accw")
            for kb in range(KB_D):
                mm = nc.tensor.matmul(accw, wo_sb[:, kb, :], ogt[:, kb, :],
                                      start=(kb == 0), stop=(kb == KB_D - 1))
                if kb == 0 and anchor is not None:
                    # ordering-only anchor: keep the scheduler from hoisting
                    # these gather-dependent matmuls into the attention
                    # stream, where an unmet collective dep would stall the
                    # in-order PE queue (cross-core skew is unmodeled).
                    bass_rust.add_dep_helper(mm.ins, anchor.ins, sync=False,
                                             reason="outproj anchor")
            osb2 = opl.tile([128, 512], F32, name="osb2")
            nc.vector.tensor_scalar_add(osb2, accw, bo_sb[:, 0:1])
            nc.sync.dma_start(out=out.ap()[:, qsl_g], in_=osb2)

        # ---------- scheduler ----------
        fillers = deque()
        pending_epi = [None]
        gather_deps = {}
        last_av = {}
        last_dps = [None]

        def pull(n=1):
            for _ in range(n):
                while fillers:
                    try:
                        next(fillers[0])
                        break
                    except StopIteration:
                        fillers.popleft()
                else:
                    return

        def attn_chunk(b, qc):
            cci = b * QC + qc
            qsl = slice(qc * 512, (qc + 1) * 512)
            po = ps.tile([128, 512], F32, tag="po", bufs=2, name="po")
            acc_v = dp.tile([128, 1024], F16, tag="av", name="acc_v")
            ets = {}

            def sc(kb):
                s_ps = ps.tile([128, 1024], F32, tag="aps", name="s_ps")
                ksl = slice(kb * 128, (kb + 1) * 128)
                for h in range(HPC):
                    hsl = slice(h * 64, (h + 1) * 64)
                    mm = nc.tensor.matmul(
                        s_ps[:, h * 512:(h + 1) * 512],
                        kT[b][hsl, ksl], qT[b][hsl, qsl],
                        start=True, stop=True, tile_position=(h * 64, 0))
                    if h == 0:
                        guard(mm, ('k', b, kb // 4))
                        guard(mm, ('q', b, qc))
                et = ep.tile([128, 1024], F16, tag="et", name="et")
                nc.scalar.activation(out=et, in_=s_ps,
                                     func=mybir.ActivationFunctionType.Exp,
                                     scale=SCALE)
                ets[kb] = et

            def av(kb):
                et = ets[kb]
                mm = nc.tensor.matmul(po[0:64, :], v2[b][:, kb, 0:64],
                                      et[:, 0:512],
                                      start=(kb == 0), stop=(kb == KBS - 1),
                                      tile_position=(0, 0))
                guard(mm, ('v2', b, kb))
                mm2 = nc.tensor.matmul(po[64:128, :], v2[b][:, kb, 64:128],
                                       et[:, 512:1024],
                                       start=(kb == 0), stop=(kb == KBS - 1),
                                       tile_position=(0, 64))
                last_av[cci] = mm2

            def dacc(kb):
                # denominator accumulation, DVE only: GpSimd must stay out
                # of the attention flow — a collective trigger blocks its
                # queue while the CC core is busy, and any attention-coupled
                # GpSimd op would stall et recycling behind it. Emitted one
                # kb late so its wait on exp(kb) never head-of-line-blocks
                # the DVE FIFO.
                et = ets.pop(kb)
                if kb == 0:
                    nc.vector.tensor_copy(acc_v, et)
                else:
                    nc.vector.tensor_add(acc_v, acc_v, et)

            sc(0)
            pull(2)
            sc(1)
            # previous chunk's epilogue: its waits (denominator fold on DVE,
            # recip, broadcasts) now overlap this chunk's fresh work instead
            # of stalling the in-order PE queue at the boundary.
            if pending_epi[0] is not None:
                pending_epi[0]()
                pending_epi[0] = None
            pull(2)
            for kb in range(KBS):
                if kb + 2 < KBS:
                    sc(kb + 2)
                av(kb)
                if kb >= 1:
                    dacc(kb - 1)
                # chunks 0-1 must pull hard: b0's filler V-chain/qT groups
                # have to be EMITTED before the avs/scores that read them.
                pull((2 if kb < 12 else 1) if cci < 2 else 1)
            dacc(KBS - 1)

            def epilogue():
                # The dummy aps allocation keeps the next chunk's sc(0) off
                # the buffer the recip is still reading.
                ps.tile([128, 1024], F32, tag="aps", name="dummy")
                d_ps = ps.tile([128, 1024], F32, tag="aps", name="d_ps")
                for half in range(2):
                    hsl = slice(half * 512, (half + 1) * 512)
                    last_dps[0] = nc.tensor.matmul(
                        d_ps[0:1, hsl], ones_sb, acc_v[:, hsl],
                        start=True, stop=True)
                rec = npl.tile([1, 1024], F32, tag="rec", bufs=2, name="rec")
                nc.vector.reciprocal_approx_fast(rec, d_ps[0:1, :])
                bcb = npl.tile([64, 1024], F32, tag="bc", name="bcb")
                nc.gpsimd.partition_broadcast(bcb[:, 0:512], rec[0:1, 0:512],
                                              channels=64)
                nc.gpsimd.partition_broadcast(bcb[:, 512:1024],
                                              rec[0:1, 512:1024], channels=64)
                with nc.allow_low_precision(reason="softmax normalize"):
                    nc.vector.tensor_mul(osb_st[0:64, cci, :], po[0:64, :],
                                         bcb[:, 0:512])
                    nc.vector.tensor_mul(osb_st[64:128, cci, :],
                                         po[64:128, :], bcb[:, 512:1024])

                gi, off = g_of[cci]
                d = nc.sync.dma_start(
                    out=o_loc[gi].ap()[:, off * 512:(off + 1) * 512],
                    in_=osb_st[:, cci, :])
                gather_deps.setdefault(gi, []).append(d.ins)
                if off == len(GATHERS[gi]) - 1:
                    cc = nc.gpsimd.collective_compute(
                        "AllGather", mybir.AluOpType.bypass,
                        replica_groups=[list(range(NCORES))],
                        ins=[o_loc[gi].ap()], outs=[o_gat[gi].ap()])
                    for dd in gather_deps[gi]:
                        bass_rust.add_dep_helper(cc.ins, dd, sync=True,
                                                 reason="cc after o_loc")
                    cc_insts[gi] = cc
                    for c2 in GATHERS[gi]:
                        ogt_dma(c2)

            pending_epi[0] = epilogue

        # prologue: b0 projections up to attn-ready, then chunk-major loop
        emit_x_dma(0, 0)
        pg0, pg1 = proj_gen(0), proj_gen(1)
        for m in pg0:
            if m == "attn_ready":
                break
        fillers.append(pg0)

        for ci in range(NCH):
            cur_ci[0] = ci
            b, qc = divmod(ci, QC)
            attn_chunk(b, qc)
            if ci == 0:
                fillers.append(pg1)

        # final chunk epilogue, drain fillers, then all out-projections as a
        # tail anchored after the last attention block: early chunks' data
        # is long gathered, and the last chunks' matmuls overlap the final
        # gathers' flight.
        cur_ci[0] = NCH
        pending_epi[0]()
        pending_epi[0] = None
        while fillers:
            pull()
        # all out-projections anchored AFTER the final chunk's denominator
        # matmuls: the last epilogue (whose o_loc dma gates the final gather
        # trigger) must not be delayed by out-proj PE work; the early
        # chunks' out-projs then run during the final gather's flight.
        for cci in range(NCH):
            outproj_mms(cci, anchor=last_dps[0])

    nc.finalize()
    return nc


def _tile_w(w, np_dt):
    # [D, W_SL] -> [128, KB_D*W_SL] matching sbuf tile [128, kb, m]
    return np.ascontiguousarray(
        w.reshape(KB_D, 128, W_SL).transpose(1, 0, 2).reshape(128, KB_D * W_SL)
    ).astype(np_dt)


def kernel(x, Wq, Wk, Wv, Wo, bo):
    import ml_dtypes
    np_dt = ml_dtypes.bfloat16
    x = np.asarray(x, dtype=np.float32)
    Wq = np.asarray(Wq, dtype=np.float32)
    Wk = np.asarray(Wk, dtype=np.float32)
    Wv = np.asarray(Wv, dtype=np.float32)
    Wo = np.asarray(Wo, dtype=np.float32)
    bo = np.asarray(bo, dtype=np.float32)

    if "nc" not in _NC_CACHE:
        _NC_CACHE["nc"] = build_nc()
    nc = _NC_CACHE["nc"]

    xt = np.ascontiguousarray(x.reshape(T, D).T).astype(np_dt)  # [D, T]
    xt0 = np.ascontiguousarray(
        xt[:, 0:512].reshape(KB_D, 128, 512).transpose(1, 0, 2).reshape(
            128, KB_D * 512))
    in_maps = []
    for c in range(NCORES):
        csl = slice(c * W_SL, (c + 1) * W_SL)
        in_maps.append({
            "xt": xt,
            "xt0": xt0,
            "wq": _tile_w(Wq[:, csl], np_dt),
            "wk": _tile_w(Wk[:, csl], np_dt),
            "wv": _tile_w(Wv[:, csl], np_dt),
            "wo": _tile_w(Wo[:, csl], np_dt),
            "bo": np.ascontiguousarray(bo[csl]).reshape(W_SL, 1),
        })
    res = run_bass_kernel_spmd(nc, in_maps, core_ids=list(range(NCORES)))
    LAST_RESULT["exec_time_ns"] = res.exec_time_ns
    LAST_RESULT["scope_times"] = res.per_core_scope_times
    LAST_RESULT["trace"] = res.instructions_and_trace[1] if res.instructions_and_trace else None
    out_t = np.concatenate([res.results[c]["out"] for c in range(NCORES)], axis=0)
    return np.ascontiguousarray(out_t.T).reshape(B, S, D)


LAST_RESULT = {}
